# revision 35
# baseline (speedup 1.0000x reference)
"""GRU-style GNN message-passing kernel for Trainium2 (8 NeuronCores, SPMD).

Reference computation (per node b, features 256, 8 neighbors):
    xr = x @ Wir.T + bir
    hr_n = hs_n @ Whr.T + bhr
    r_n = sigmoid(xr + hr_n)
    z = sigmoid(x @ Wiz.T + biz + h_sum @ Whz.T + bhz)
    s = sum_n r_n * hs_n
    n = tanh(x @ Win.T + bin + s @ Whn.T + bhn)
    out = (1 - z) * n + z * h_sum

Strategy: data-parallel over the node dim B=32768 across 8 cores (4096
rows each). Per core the 4096 rows are processed as 8 subchunks of 512
grouped into chunks of width [512,512,1024,1024,512,512] -- small edge
chunks shorten the DMA-bound head and the drain tail, wide middle
chunks amortize per-instruction overhead. Everything on-chip is bf16
(fp32 PSUM accumulation) in feature-major layout. Host pre-packs every
subchunk contiguously in HBM so each DMA moves 2-16KB/partition lines.
Engine placement:
  - PE: all linear-layer matmuls in [128,512] PSUM-bank groups; the
    shared (xr + b_r) term is injected into each neighbor's PSUM group
    via an identity matmul; n-gate emits Win matmuls before Whn ones so
    PE has work while the neighbor sum finishes.
  - ACT: sigmoid/tanh (per-feature bias), xr PSUM drain (+b_r).
  - DVE: r*hs products in place in the hs tile and a linear running sum
    (short dependency tail), plus the final combine out = n + z*(h-n),
    all bf16 2x mode, split by feature half to shorten the tail.
Gate work (z, xr) and the first two neighbors of chunk c+1 are emitted
interleaved into chunk c's neighbor loop, so the two 4-bank PSUM slots
rotate without stalling the PE queue.
"""

import sys
import numpy as np
from contextlib import ExitStack

sys.path.insert(0, "/opt/trn_rl_repo")

import ml_dtypes
import concourse.bacc as bacc
import concourse.tile as tile
from concourse import mybir
from concourse.bass_utils import run_bass_kernel_spmd

F32 = mybir.dt.float32
BF16 = mybir.dt.bfloat16
BF_NP = ml_dtypes.bfloat16

N_NEIGH, B, IN, H = 8, 32768, 256, 256
M = 8                    # cores
BL = B // M              # rows per core (4096)
SW = 512                 # subchunk width
NSUB = BL // SW          # 8 subchunks per core
CHUNKS = [(0, 1), (1, 2), (3, 2), (5, 2), (7, 1)]  # (start, n_subs)
NC = len(CHUNKS)

_cached = None  # compiled program, reused across kernel() calls


def _build():
    nc = bacc.Bacc("TRN2", target_bir_lowering=False, debug=False, num_devices=M)

    # per-subchunk packed inputs (see _prep_inputs for layouts)
    xS = nc.dram_tensor("xS", [NSUB, 128, 2 * SW], BF16, kind="ExternalInput").ap()
    hS = nc.dram_tensor("hS", [NSUB, 128, 2 * SW], BF16, kind="ExternalInput").ap()
    hsS = nc.dram_tensor("hsS", [NSUB, 128, 2 * N_NEIGH * SW], BF16,
                         kind="ExternalInput").ap()
    # all bf16 constants in one block: identity [128,128] first, then 6
    # weights x 2 contraction-row blocks of [128,256] each, ordered so the
    # head-critical ones (wiz, whz, wir) come before whr/win/whn
    wpack = nc.dram_tensor("wpack", [128, 128 + 6 * 512], BF16,
                           kind="ExternalInput").ap()
    # bias pack: col f*3+j holds feature-chunk f of (b_r, b_z, b_n)[j]
    biasp = nc.dram_tensor("biasp", [128, 6], F32, kind="ExternalInput").ap()
    # output: chunk-major blocks, (f, ch, j) within each chunk block
    outF = nc.dram_tensor("outF", [128, 2 * BL], BF16, kind="ExternalOutput").ap()

    with tile.TileContext(nc) as tc, ExitStack() as ctx:
        const_pool = ctx.enter_context(tc.tile_pool(name="const", bufs=1))
        x_pool = ctx.enter_context(tc.tile_pool(name="x", bufs=2))
        h_pool = ctx.enter_context(tc.tile_pool(name="h", bufs=2))
        hs_pool = ctx.enter_context(tc.tile_pool(name="hs", bufs=2))
        xr_pool = ctx.enter_context(tc.tile_pool(name="xr", bufs=2))
        z_pool = ctx.enter_context(tc.tile_pool(name="z", bufs=2))
        r_pool = ctx.enter_context(tc.tile_pool(name="r", bufs=4))
        n_pool = ctx.enter_context(tc.tile_pool(name="n", bufs=2))
        d_pool = ctx.enter_context(tc.tile_pool(name="d", bufs=2))
        o_pool = ctx.enter_context(tc.tile_pool(name="o", bufs=2))
        ps_pool = ctx.enter_context(tc.tile_pool(name="ps", bufs=2, space="PSUM"))

        # --- constants: two bf16 piece DMAs (head-critical piece first so
        # warmup + z/xr gates start ~1.4us earlier) + one f32 bias DMA ---
        HEAD_COLS = 128 + 3 * 512   # id + wiz + whz + wir
        wpk_t = const_pool.tile([128, 128 + 6 * 512], BF16, tag="wpack",
                                name="wpk_t")
        nc.sync.dma_start(out=wpk_t[:, 0:HEAD_COLS], in_=wpack[:, 0:HEAD_COLS])
        nc.sync.dma_start(out=wpk_t[:, HEAD_COLS:], in_=wpack[:, HEAD_COLS:])
        bias_t = const_pool.tile([128, 6], F32, tag="biasp", name="bias_t")
        nc.sync.dma_start(out=bias_t[:, :], in_=biasp[:, :])

        W_ORDER = ("wiz", "whz", "wir", "whr", "win", "whn")

        def wcol(w, cb, f):       # stationary [128,128]: contract block cb, out block f
            # wpack[p, 128 + (wi*2+cb)*256 + m] = W.T[cb*128+p, m]
            base = 128 + (W_ORDER.index(w) * 2 + cb) * 256 + f * 128
            return wpk_t[:, base:base + 128]

        id_t = wpk_t[:, 0:128]

        # live tiles per chunk index
        xt = [None] * NC
        ht = [None] * NC
        hsc = [None] * NC
        xrt = [None] * NC
        zt = [None] * NC

        def nsubs(c):
            return CHUNKS[c][1]

        def dma_in(c, use_scalar=False):
            ns = nsubs(c)
            start = CHUNKS[c][0]
            xt[c] = x_pool.tile([128, 2 * SW * 2], BF16, tag="x", name=f"x_{c}")
            ht[c] = h_pool.tile([128, 2 * SW * 2], BF16, tag="h", name=f"h_{c}")
            hsc[c] = hs_pool.tile([128, 2 * N_NEIGH * SW * 2], BF16, tag="hs",
                                  name=f"hs_{c}")
            for ch in range(ns):
                s = start + ch
                nc.sync.dma_start(out=xt[c][:, ch * 1024:(ch + 1) * 1024],
                                  in_=xS[s])
                (nc.scalar if use_scalar else nc.sync).dma_start(
                    out=ht[c][:, ch * 1024:(ch + 1) * 1024], in_=hS[s])
                # hs subchunk (2MB) split in two for progressive neighbor deps
                eng2 = nc.scalar if use_scalar else nc.sync
                nc.sync.dma_start(
                    out=hsc[c][:, ch * 8192:ch * 8192 + 4096],
                    in_=hsS[s][:, 0:4096])
                eng2.dma_start(
                    out=hsc[c][:, ch * 8192 + 4096:(ch + 1) * 8192],
                    in_=hsS[s][:, 4096:8192])

        def mslice(t, cb, ch):    # moving [128,512]: x/h tiles, (ch, cb, j) layout
            base = ch * 1024 + cb * 512
            return t[:, base:base + 512]

        def hslice(c, n, cb, ch):  # moving [128,512] of neighbor n
            base = ch * 8192 + (n * 2 + cb) * 512
            return hsc[c][:, base:base + 512]

        def q_of(p, cw, f, ch):    # PSUM quarter (f, ch) of a [128, 2*cw] tile
            base = f * cw + ch * 512
            return p[:, base:base + 512]

        def gates_zx(c, warm=False):
            """z-gate and xr for chunk c: matmuls + ACT/DVE drains."""
            ns = nsubs(c)
            cw = ns * SW
            pz = ps_pool.tile([128, 2048], F32, tag="ps", name=f"pz_{c}")
            if warm:
                # head-only: 8 matmuls on the weight pack (the first DMA to
                # land) into pz banks the real z-gate never touches, so the
                # PE is busy+warm (HAM K=8/8) through the chunk-0 hs wait
                for g in range(2):
                    q = pz[:, 1024 + g * 512:1024 + (g + 1) * 512]
                    for k in range(4):
                        nc.tensor.matmul(q, id_t,
                                         wpk_t[:, 128 + k * 512:
                                                128 + (k + 1) * 512],
                                         start=(k == 0), stop=(k == 3))
            for f in range(2):
                for ch in range(ns):
                    q = q_of(pz, cw, f, ch)
                    nc.tensor.matmul(q, wcol("wiz", 0, f), mslice(xt[c], 0, ch),
                                     start=True, stop=False)
                    nc.tensor.matmul(q, wcol("wiz", 1, f), mslice(xt[c], 1, ch),
                                     start=False, stop=False)
                    nc.tensor.matmul(q, wcol("whz", 0, f), mslice(ht[c], 0, ch),
                                     start=False, stop=False)
                    nc.tensor.matmul(q, wcol("whz", 1, f), mslice(ht[c], 1, ch),
                                     start=False, stop=True)
            zt[c] = z_pool.tile([128, 2048], BF16, tag="z", name=f"z_{c}")
            for f in range(2):
                nc.scalar.activation(zt[c][:, f * cw:(f + 1) * cw],
                                     pz[:, f * cw:(f + 1) * cw],
                                     mybir.ActivationFunctionType.Sigmoid,
                                     bias=bias_t[:, f * 3 + 1:f * 3 + 2])

            pxr = ps_pool.tile([128, 2048], F32, tag="ps", name=f"pxr_{c}")
            for f in range(2):
                for ch in range(ns):
                    q = q_of(pxr, cw, f, ch)
                    nc.tensor.matmul(q, wcol("wir", 0, f), mslice(xt[c], 0, ch),
                                     start=True, stop=False)
                    nc.tensor.matmul(q, wcol("wir", 1, f), mslice(xt[c], 1, ch),
                                     start=False, stop=True)
            xrt[c] = xr_pool.tile([128, 2048], BF16, tag="xr", name=f"xr_{c}")
            # drain on DVE (has slack) instead of ACT: keeps the ACT queue
            # free for the sigmoid cadence at gate boundaries
            with nc.allow_low_precision(reason="bf16 xr"):
                for f in range(2):
                    nc.vector.tensor_scalar_add(xrt[c][:, f * cw:(f + 1) * cw],
                                                pxr[:, f * cw:(f + 1) * cw],
                                                bias_t[:, f * 3:f * 3 + 1])

        def neigh(c, n):
            """r_n = sigmoid(Whr@hs_n + xr), product into hsc, running sum."""
            ns = nsubs(c)
            cw = ns * SW
            pr = ps_pool.tile([128, 2048], F32, tag="ps", name=f"pr{n}_{c}")
            for f in range(2):
                for ch in range(ns):
                    q = q_of(pr, cw, f, ch)
                    nc.tensor.matmul(q, wcol("whr", 0, f), hslice(c, n, 0, ch),
                                     start=True, stop=False)
                    nc.tensor.matmul(q, wcol("whr", 1, f), hslice(c, n, 1, ch),
                                     start=False, stop=False)
                    nc.tensor.matmul(q, id_t,
                                     xrt[c][:, f * cw + ch * 512:
                                            f * cw + ch * 512 + 512],
                                     start=False, stop=True)
            rc = r_pool.tile([128, 2048], BF16, tag="r", name=f"r{n}_{c}")
            nc.scalar.activation(rc[:, 0:2 * cw], pr[:, 0:2 * cw],
                                 mybir.ActivationFunctionType.Sigmoid)
            # rc is (f, ch, j); hs neighbor block is (ch, cb, j): use 4D APs
            rc4 = rc[:, 0:2 * cw].rearrange("p (f ch j) -> p ch f j",
                                            f=2, ch=ns)
            hs4 = hsc[c][:, 0:ns * 8192].rearrange(
                "p (ch nn cb j) -> p nn ch cb j", ch=ns, nn=N_NEIGH, cb=2)
            hs_n = hs4[:, n]
            nc.vector.tensor_mul(hs_n, rc4, hs_n)
            if n > 0:
                acc = hs4[:, 0]
                with nc.allow_low_precision(reason="bf16 neighbor sums"):
                    nc.vector.tensor_add(acc, acc, hs_n)

        def ngate(c):
            """n gate + combine + output DMA for chunk c."""
            ns = nsubs(c)
            cw = ns * SW
            colbase = 2 * CHUNKS[c][0] * SW
            pn = ps_pool.tile([128, 2048], F32, tag="ps", name=f"pn_{c}")
            # all Win matmuls first: they only need x, so PE has work while
            # the DVE accumulation of s finishes
            for f in range(2):
                for ch in range(ns):
                    q = q_of(pn, cw, f, ch)
                    nc.tensor.matmul(q, wcol("win", 0, f), mslice(xt[c], 0, ch),
                                     start=True, stop=False)
                    nc.tensor.matmul(q, wcol("win", 1, f), mslice(xt[c], 1, ch),
                                     start=False, stop=False)
            for f in range(2):
                for ch in range(ns):
                    q = q_of(pn, cw, f, ch)
                    nc.tensor.matmul(q, wcol("whn", 0, f), hslice(c, 0, 0, ch),
                                     start=False, stop=False)
                    nc.tensor.matmul(q, wcol("whn", 1, f), hslice(c, 0, 1, ch),
                                     start=False, stop=True)
            nt = n_pool.tile([128, 2048], BF16, tag="n", name=f"n_{c}")
            dt_ = d_pool.tile([128, 2048], BF16, tag="d", name=f"d_{c}")
            ot = o_pool.tile([128, 2048], BF16, tag="o", name=f"o_{c}")
            # per f-half: tanh then bf16 combine out = n + z*(h - n) on DVE,
            # then its output DMA -- halves the exposed tail latency.
            # h is (ch, f, j)-major, the rest (f, ch, j): rearranged APs.
            for f in range(2):
                fs = slice(f * cw, (f + 1) * cw)
                nc.scalar.activation(nt[:, fs], pn[:, fs],
                                     mybir.ActivationFunctionType.Tanh,
                                     bias=bias_t[:, f * 3 + 2:f * 3 + 3])
                htf = ht[c][:, 0:2 * cw].rearrange(
                    "p (ch f j) -> p f ch j", ch=ns, f=2)[:, f]
                ntf = nt[:, fs].rearrange("p (ch j) -> p ch j", ch=ns)
                dtf = dt_[:, fs].rearrange("p (ch j) -> p ch j", ch=ns)
                ztf = zt[c][:, fs].rearrange("p (ch j) -> p ch j", ch=ns)
                otf = ot[:, fs].rearrange("p (ch j) -> p ch j", ch=ns)
                with nc.allow_low_precision(reason="bf16 combine"):
                    nc.vector.tensor_sub(dtf, htf, ntf)
                    nc.vector.tensor_mul(dtf, ztf, dtf)
                    nc.vector.tensor_add(otf, ntf, dtf)
                nc.sync.dma_start(
                    out=outF[:, colbase + f * cw:colbase + (f + 1) * cw],
                    in_=ot[:, fs])

        # --- software-pipelined emission ---
        dma_in(0, use_scalar=True)
        gates_zx(0, warm=True)
        neigh(0, 0)
        neigh(0, 1)
        for c in range(NC):
            neigh(c, 2)
            neigh(c, 3)
            neigh(c, 4)
            neigh(c, 5)
            if c + 1 < NC:
                dma_in(c + 1, use_scalar=(c == 0))
                gates_zx(c + 1)
            neigh(c, 6)
            neigh(c, 7)
            if c + 1 < NC:
                neigh(c + 1, 0)
            ngate(c)
            if c + 1 < NC:
                neigh(c + 1, 1)

    nc.compile()
    return nc


def _prep_inputs(x, h_sum, hs, Wir, bir, Whr, bhr, Wiz, biz, Whz, bhz,
                 Win, bin_, Whn, bhn):
    """Shard + transpose + subchunk-pack to per-core input maps (all bf16)."""
    f32 = np.float32

    def sub_pack(a):  # [BL, 256] -> [NSUB, 128, 1024]; [s,p,cb*512+j]=a[s*512+j,cb*128+p]
        return np.ascontiguousarray(
            a.reshape(NSUB, SW, 2, 128).transpose(0, 3, 2, 1)
        ).reshape(NSUB, 128, 2 * SW).astype(BF_NP)

    def hs_pack(a):  # [8, BL, 256] -> [NSUB, 128, 8192]; [s,p,(n*2+cb)*512+j]
        return np.ascontiguousarray(
            a.reshape(N_NEIGH, NSUB, SW, 2, 128).transpose(1, 4, 0, 3, 2)
        ).reshape(NSUB, 128, 2 * N_NEIGH * SW).astype(BF_NP)

    # wpack[p, 128 + (wi*2+cb)*256 + m] = W.T[cb*128+p, m]; identity first
    wpack = np.empty((128, 128 + 6 * 512), BF_NP)
    wpack[:, 0:128] = np.eye(128, dtype=f32).astype(BF_NP)
    for wi, Wm in enumerate((Wiz, Whz, Wir, Whr, Win, Whn)):
        Wt = np.asarray(Wm, f32).T.astype(BF_NP)
        wpack[:, 128 + (wi * 2) * 256:128 + (wi * 2 + 1) * 256] = Wt[0:128, :]
        wpack[:, 128 + (wi * 2 + 1) * 256:128 + (wi * 2 + 2) * 256] = Wt[128:256, :]

    b_r = np.asarray(bir, f32) + np.asarray(bhr, f32)
    b_z = np.asarray(biz, f32) + np.asarray(bhz, f32)
    b_n = np.asarray(bin_, f32) + np.asarray(bhn, f32)
    biasp = np.empty((128, 6), f32)
    for f in range(2):
        biasp[:, f * 3 + 0] = b_r[f * 128:(f + 1) * 128]
        biasp[:, f * 3 + 1] = b_z[f * 128:(f + 1) * 128]
        biasp[:, f * 3 + 2] = b_n[f * 128:(f + 1) * 128]

    xf = np.asarray(x, f32)
    hf = np.asarray(h_sum, f32)
    hsf = np.asarray(hs, f32)

    in_maps = []
    for c in range(M):
        sl = slice(c * BL, (c + 1) * BL)
        m = {
            "xS": sub_pack(xf[sl]),
            "hS": sub_pack(hf[sl]),
            "hsS": hs_pack(hsf[:, sl]),
            "wpack": wpack,
            "biasp": biasp,
        }
        in_maps.append(m)
    return in_maps


def _run(inputs, trace=False, **trace_kwargs):
    global _cached
    if _cached is None:
        _cached = _build()
    nc = _cached
    in_maps = _prep_inputs(**inputs)
    res = run_bass_kernel_spmd(nc, in_maps, list(range(M)), trace=trace,
                               **trace_kwargs)
    out = np.empty((B, H), np.float32)
    for core in range(M):
        # outF [128, 2*BL]: chunk-major blocks, (f, ch, j) inside
        o = np.asarray(res.results[core]["outF"]).astype(np.float32)
        oc = np.empty((BL, 256), np.float32)
        for (start, ns) in CHUNKS:
            cb0 = 2 * start * SW
            blk = o[:, cb0:cb0 + 2 * ns * SW].reshape(128, 2, ns, SW)
            # oc[(start+ch)*SW + j, f*128 + p] = blk[p, f, ch, j]
            oc[start * SW:(start + ns) * SW, :] = (
                blk.transpose(2, 3, 1, 0).reshape(ns * SW, 256))
        out[core * BL:(core + 1) * BL, :] = oc
    return out, res


def kernel(**inputs):
    return _run(inputs)[0]


# revision 36
# speedup vs baseline: 1.0180x; 1.0180x over previous
"""GRU-style GNN message-passing kernel for Trainium2 (8 NeuronCores, SPMD).

Reference computation (per node b, features 256, 8 neighbors):
    xr = x @ Wir.T + bir
    hr_n = hs_n @ Whr.T + bhr
    r_n = sigmoid(xr + hr_n)
    z = sigmoid(x @ Wiz.T + biz + h_sum @ Whz.T + bhz)
    s = sum_n r_n * hs_n
    n = tanh(x @ Win.T + bin + s @ Whn.T + bhn)
    out = (1 - z) * n + z * h_sum

Strategy: data-parallel over the node dim B=32768 across 8 cores (4096
rows each). Per core the 4096 rows are processed as 8 subchunks of 512
grouped into chunks of width [512,512,1024,1024,512,512] -- small edge
chunks shorten the DMA-bound head and the drain tail, wide middle
chunks amortize per-instruction overhead. Everything on-chip is bf16
(fp32 PSUM accumulation) in feature-major layout. Host pre-packs every
subchunk contiguously in HBM so each DMA moves 2-16KB/partition lines.
Engine placement:
  - PE: all linear-layer matmuls in [128,512] PSUM-bank groups; the
    shared (xr + b_r) term is injected into each neighbor's PSUM group
    via an identity matmul; n-gate emits Win matmuls before Whn ones so
    PE has work while the neighbor sum finishes.
  - ACT: sigmoid/tanh (per-feature bias), xr PSUM drain (+b_r).
  - DVE: r*hs products in place in the hs tile and a linear running sum
    (short dependency tail), plus the final combine out = n + z*(h-n),
    all bf16 2x mode, split by feature half to shorten the tail.
Gate work (z, xr) and the first two neighbors of chunk c+1 are emitted
interleaved into chunk c's neighbor loop, so the two 4-bank PSUM slots
rotate without stalling the PE queue.
"""

import sys
import numpy as np
from contextlib import ExitStack

sys.path.insert(0, "/opt/trn_rl_repo")

import ml_dtypes
import concourse.bacc as bacc
import concourse.tile as tile
from concourse import mybir
from concourse.bass_utils import run_bass_kernel_spmd

F32 = mybir.dt.float32
BF16 = mybir.dt.bfloat16
BF_NP = ml_dtypes.bfloat16

N_NEIGH, B, IN, H = 8, 32768, 256, 256
M = 8                    # cores
BL = B // M              # rows per core (4096)
SW = 512                 # subchunk width
NSUB = BL // SW          # 8 subchunks per core
CHUNKS = [(0, 1), (1, 2), (3, 2), (5, 2), (7, 1)]  # (start, n_subs)
NC = len(CHUNKS)

_cached = None  # compiled program, reused across kernel() calls


def _build():
    nc = bacc.Bacc("TRN2", target_bir_lowering=False, debug=False, num_devices=M)

    # per-subchunk packed inputs (see _prep_inputs for layouts)
    xS = nc.dram_tensor("xS", [NSUB, 128, 2 * SW], BF16, kind="ExternalInput").ap()
    hS = nc.dram_tensor("hS", [NSUB, 128, 2 * SW], BF16, kind="ExternalInput").ap()
    hsS = nc.dram_tensor("hsS", [NSUB, 128, 2 * N_NEIGH * SW], BF16,
                         kind="ExternalInput").ap()
    # all bf16 constants in one block: identity [128,128] first, then 6
    # weights x 2 contraction-row blocks of [128,256] each, ordered so the
    # head-critical ones (wiz, whz, wir) come before whr/win/whn
    wpack = nc.dram_tensor("wpack", [128, 128 + 6 * 512], BF16,
                           kind="ExternalInput").ap()
    # bias pack: col f*3+j holds feature-chunk f of (b_r, b_z, b_n)[j]
    biasp = nc.dram_tensor("biasp", [128, 6], F32, kind="ExternalInput").ap()
    # output: chunk-major blocks, (f, ch, j) within each chunk block
    outF = nc.dram_tensor("outF", [128, 2 * BL], BF16, kind="ExternalOutput").ap()

    with tile.TileContext(nc) as tc, ExitStack() as ctx:
        const_pool = ctx.enter_context(tc.tile_pool(name="const", bufs=1))
        x_pool = ctx.enter_context(tc.tile_pool(name="x", bufs=2))
        h_pool = ctx.enter_context(tc.tile_pool(name="h", bufs=2))
        hs_pool = ctx.enter_context(tc.tile_pool(name="hs", bufs=2))
        xr_pool = ctx.enter_context(tc.tile_pool(name="xr", bufs=2))
        z_pool = ctx.enter_context(tc.tile_pool(name="z", bufs=2))
        r_pool = ctx.enter_context(tc.tile_pool(name="r", bufs=4))
        n_pool = ctx.enter_context(tc.tile_pool(name="n", bufs=2))
        d_pool = ctx.enter_context(tc.tile_pool(name="d", bufs=2))
        o_pool = ctx.enter_context(tc.tile_pool(name="o", bufs=2))
        ps_pool = ctx.enter_context(tc.tile_pool(name="ps", bufs=2, space="PSUM"))

        # --- constants: one bf16 block DMA + one small f32 bias DMA ---
        wpk_t = const_pool.tile([128, 128 + 6 * 512], BF16, tag="wpack",
                                name="wpk_t")
        nc.sync.dma_start(out=wpk_t[:, :], in_=wpack[:, :])
        bias_t = const_pool.tile([128, 6], F32, tag="biasp", name="bias_t")
        nc.sync.dma_start(out=bias_t[:, :], in_=biasp[:, :])

        W_ORDER = ("wiz", "whz", "wir", "whr", "win", "whn")

        def wcol(w, cb, f):       # stationary [128,128]: contract block cb, out block f
            # wpack[p, 128 + (wi*2+cb)*256 + m] = W.T[cb*128+p, m]
            base = 128 + (W_ORDER.index(w) * 2 + cb) * 256 + f * 128
            return wpk_t[:, base:base + 128]

        id_t = wpk_t[:, 0:128]

        # live tiles per chunk index
        xt = [None] * NC
        ht = [None] * NC
        hsc = [None] * NC
        xrt = [None] * NC
        zt = [None] * NC

        def nsubs(c):
            return CHUNKS[c][1]

        def dma_in(c, use_scalar=False):
            ns = nsubs(c)
            start = CHUNKS[c][0]
            xt[c] = x_pool.tile([128, 2 * SW * 2], BF16, tag="x", name=f"x_{c}")
            ht[c] = h_pool.tile([128, 2 * SW * 2], BF16, tag="h", name=f"h_{c}")
            hsc[c] = hs_pool.tile([128, 2 * N_NEIGH * SW * 2], BF16, tag="hs",
                                  name=f"hs_{c}")
            for ch in range(ns):
                s = start + ch
                nc.sync.dma_start(out=xt[c][:, ch * 1024:(ch + 1) * 1024],
                                  in_=xS[s])
                (nc.scalar if use_scalar else nc.sync).dma_start(
                    out=ht[c][:, ch * 1024:(ch + 1) * 1024], in_=hS[s])
                # hs subchunk (2MB) split in two for progressive neighbor deps
                eng2 = nc.scalar if use_scalar else nc.sync
                nc.sync.dma_start(
                    out=hsc[c][:, ch * 8192:ch * 8192 + 4096],
                    in_=hsS[s][:, 0:4096])
                eng2.dma_start(
                    out=hsc[c][:, ch * 8192 + 4096:(ch + 1) * 8192],
                    in_=hsS[s][:, 4096:8192])

        def mslice(t, cb, ch):    # moving [128,512]: x/h tiles, (ch, cb, j) layout
            base = ch * 1024 + cb * 512
            return t[:, base:base + 512]

        def hslice(c, n, cb, ch):  # moving [128,512] of neighbor n
            base = ch * 8192 + (n * 2 + cb) * 512
            return hsc[c][:, base:base + 512]

        def q_of(p, cw, f, ch):    # PSUM quarter (f, ch) of a [128, 2*cw] tile
            base = f * cw + ch * 512
            return p[:, base:base + 512]

        def gates_zx(c, warm=False):
            """z-gate and xr for chunk c: matmuls + ACT/DVE drains."""
            ns = nsubs(c)
            cw = ns * SW
            pz = ps_pool.tile([128, 2048], F32, tag="ps", name=f"pz_{c}")
            if warm:
                # head-only: 8 matmuls on the weight pack (the first DMA to
                # land) into pz banks the real z-gate never touches, so the
                # PE is busy+warm (HAM K=8/8) through the chunk-0 hs wait
                for g in range(2):
                    q = pz[:, 1024 + g * 512:1024 + (g + 1) * 512]
                    for k in range(4):
                        nc.tensor.matmul(q, id_t,
                                         wpk_t[:, 128 + k * 512:
                                                128 + (k + 1) * 512],
                                         start=(k == 0), stop=(k == 3))
            for f in range(2):
                for ch in range(ns):
                    q = q_of(pz, cw, f, ch)
                    nc.tensor.matmul(q, wcol("wiz", 0, f), mslice(xt[c], 0, ch),
                                     start=True, stop=False)
                    nc.tensor.matmul(q, wcol("wiz", 1, f), mslice(xt[c], 1, ch),
                                     start=False, stop=False)
                    nc.tensor.matmul(q, wcol("whz", 0, f), mslice(ht[c], 0, ch),
                                     start=False, stop=False)
                    nc.tensor.matmul(q, wcol("whz", 1, f), mslice(ht[c], 1, ch),
                                     start=False, stop=True)
            zt[c] = z_pool.tile([128, 2048], BF16, tag="z", name=f"z_{c}")
            for f in range(2):
                nc.scalar.activation(zt[c][:, f * cw:(f + 1) * cw],
                                     pz[:, f * cw:(f + 1) * cw],
                                     mybir.ActivationFunctionType.Sigmoid,
                                     bias=bias_t[:, f * 3 + 1:f * 3 + 2])

            pxr = ps_pool.tile([128, 2048], F32, tag="ps", name=f"pxr_{c}")
            for f in range(2):
                for ch in range(ns):
                    q = q_of(pxr, cw, f, ch)
                    nc.tensor.matmul(q, wcol("wir", 0, f), mslice(xt[c], 0, ch),
                                     start=True, stop=False)
                    nc.tensor.matmul(q, wcol("wir", 1, f), mslice(xt[c], 1, ch),
                                     start=False, stop=True)
            xrt[c] = xr_pool.tile([128, 2048], BF16, tag="xr", name=f"xr_{c}")
            # drain on DVE (has slack) instead of ACT: keeps the ACT queue
            # free for the sigmoid cadence at gate boundaries
            with nc.allow_low_precision(reason="bf16 xr"):
                for f in range(2):
                    nc.vector.tensor_scalar_add(xrt[c][:, f * cw:(f + 1) * cw],
                                                pxr[:, f * cw:(f + 1) * cw],
                                                bias_t[:, f * 3:f * 3 + 1])

        def neigh(c, n):
            """r_n = sigmoid(Whr@hs_n + xr), product into hsc, running sum."""
            ns = nsubs(c)
            cw = ns * SW
            pr = ps_pool.tile([128, 2048], F32, tag="ps", name=f"pr{n}_{c}")
            for f in range(2):
                for ch in range(ns):
                    q = q_of(pr, cw, f, ch)
                    nc.tensor.matmul(q, wcol("whr", 0, f), hslice(c, n, 0, ch),
                                     start=True, stop=False)
                    nc.tensor.matmul(q, wcol("whr", 1, f), hslice(c, n, 1, ch),
                                     start=False, stop=False)
                    nc.tensor.matmul(q, id_t,
                                     xrt[c][:, f * cw + ch * 512:
                                            f * cw + ch * 512 + 512],
                                     start=False, stop=True)
            rc = r_pool.tile([128, 2048], BF16, tag="r", name=f"r{n}_{c}")
            nc.scalar.activation(rc[:, 0:2 * cw], pr[:, 0:2 * cw],
                                 mybir.ActivationFunctionType.Sigmoid)
            # rc is (f, ch, j); hs neighbor block is (ch, cb, j): use 4D APs
            rc4 = rc[:, 0:2 * cw].rearrange("p (f ch j) -> p ch f j",
                                            f=2, ch=ns)
            hs4 = hsc[c][:, 0:ns * 8192].rearrange(
                "p (ch nn cb j) -> p nn ch cb j", ch=ns, nn=N_NEIGH, cb=2)
            hs_n = hs4[:, n]
            nc.vector.tensor_mul(hs_n, rc4, hs_n)
            if n > 0:
                acc = hs4[:, 0]
                with nc.allow_low_precision(reason="bf16 neighbor sums"):
                    nc.vector.tensor_add(acc, acc, hs_n)

        def ngate(c):
            """n gate + combine + output DMA for chunk c."""
            ns = nsubs(c)
            cw = ns * SW
            colbase = 2 * CHUNKS[c][0] * SW
            pn = ps_pool.tile([128, 2048], F32, tag="ps", name=f"pn_{c}")
            # all Win matmuls first: they only need x, so PE has work while
            # the DVE accumulation of s finishes
            for f in range(2):
                for ch in range(ns):
                    q = q_of(pn, cw, f, ch)
                    nc.tensor.matmul(q, wcol("win", 0, f), mslice(xt[c], 0, ch),
                                     start=True, stop=False)
                    nc.tensor.matmul(q, wcol("win", 1, f), mslice(xt[c], 1, ch),
                                     start=False, stop=False)
            for f in range(2):
                for ch in range(ns):
                    q = q_of(pn, cw, f, ch)
                    nc.tensor.matmul(q, wcol("whn", 0, f), hslice(c, 0, 0, ch),
                                     start=False, stop=False)
                    nc.tensor.matmul(q, wcol("whn", 1, f), hslice(c, 0, 1, ch),
                                     start=False, stop=True)
            nt = n_pool.tile([128, 2048], BF16, tag="n", name=f"n_{c}")
            dt_ = d_pool.tile([128, 2048], BF16, tag="d", name=f"d_{c}")
            ot = o_pool.tile([128, 2048], BF16, tag="o", name=f"o_{c}")
            # per f-half: tanh then bf16 combine out = n + z*(h - n) on DVE,
            # then its output DMA -- halves the exposed tail latency.
            # h is (ch, f, j)-major, the rest (f, ch, j): rearranged APs.
            for f in range(2):
                fs = slice(f * cw, (f + 1) * cw)
                nc.scalar.activation(nt[:, fs], pn[:, fs],
                                     mybir.ActivationFunctionType.Tanh,
                                     bias=bias_t[:, f * 3 + 2:f * 3 + 3])
                htf = ht[c][:, 0:2 * cw].rearrange(
                    "p (ch f j) -> p f ch j", ch=ns, f=2)[:, f]
                ntf = nt[:, fs].rearrange("p (ch j) -> p ch j", ch=ns)
                dtf = dt_[:, fs].rearrange("p (ch j) -> p ch j", ch=ns)
                ztf = zt[c][:, fs].rearrange("p (ch j) -> p ch j", ch=ns)
                otf = ot[:, fs].rearrange("p (ch j) -> p ch j", ch=ns)
                with nc.allow_low_precision(reason="bf16 combine"):
                    nc.vector.tensor_sub(dtf, htf, ntf)
                    nc.vector.tensor_mul(dtf, ztf, dtf)
                    nc.vector.tensor_add(otf, ntf, dtf)
                nc.sync.dma_start(
                    out=outF[:, colbase + f * cw:colbase + (f + 1) * cw],
                    in_=ot[:, fs])

        # --- software-pipelined emission ---
        dma_in(0, use_scalar=True)
        gates_zx(0, warm=True)
        neigh(0, 0)
        neigh(0, 1)
        for c in range(NC):
            neigh(c, 2)
            neigh(c, 3)
            neigh(c, 4)
            neigh(c, 5)
            if c + 1 < NC:
                dma_in(c + 1, use_scalar=(c == 0))
                gates_zx(c + 1)
            neigh(c, 6)
            neigh(c, 7)
            if c + 1 < NC:
                neigh(c + 1, 0)
            ngate(c)
            if c + 1 < NC:
                neigh(c + 1, 1)

    nc.compile()
    return nc


def _prep_inputs(x, h_sum, hs, Wir, bir, Whr, bhr, Wiz, biz, Whz, bhz,
                 Win, bin_, Whn, bhn):
    """Shard + transpose + subchunk-pack to per-core input maps (all bf16)."""
    f32 = np.float32

    def sub_pack(a):  # [BL, 256] -> [NSUB, 128, 1024]; [s,p,cb*512+j]=a[s*512+j,cb*128+p]
        return np.ascontiguousarray(
            a.reshape(NSUB, SW, 2, 128).transpose(0, 3, 2, 1)
        ).reshape(NSUB, 128, 2 * SW).astype(BF_NP)

    def hs_pack(a):  # [8, BL, 256] -> [NSUB, 128, 8192]; [s,p,(n*2+cb)*512+j]
        return np.ascontiguousarray(
            a.reshape(N_NEIGH, NSUB, SW, 2, 128).transpose(1, 4, 0, 3, 2)
        ).reshape(NSUB, 128, 2 * N_NEIGH * SW).astype(BF_NP)

    # wpack[p, 128 + (wi*2+cb)*256 + m] = W.T[cb*128+p, m]; identity first
    wpack = np.empty((128, 128 + 6 * 512), BF_NP)
    wpack[:, 0:128] = np.eye(128, dtype=f32).astype(BF_NP)
    for wi, Wm in enumerate((Wiz, Whz, Wir, Whr, Win, Whn)):
        Wt = np.asarray(Wm, f32).T.astype(BF_NP)
        wpack[:, 128 + (wi * 2) * 256:128 + (wi * 2 + 1) * 256] = Wt[0:128, :]
        wpack[:, 128 + (wi * 2 + 1) * 256:128 + (wi * 2 + 2) * 256] = Wt[128:256, :]

    b_r = np.asarray(bir, f32) + np.asarray(bhr, f32)
    b_z = np.asarray(biz, f32) + np.asarray(bhz, f32)
    b_n = np.asarray(bin_, f32) + np.asarray(bhn, f32)
    biasp = np.empty((128, 6), f32)
    for f in range(2):
        biasp[:, f * 3 + 0] = b_r[f * 128:(f + 1) * 128]
        biasp[:, f * 3 + 1] = b_z[f * 128:(f + 1) * 128]
        biasp[:, f * 3 + 2] = b_n[f * 128:(f + 1) * 128]

    xf = np.asarray(x, f32)
    hf = np.asarray(h_sum, f32)
    hsf = np.asarray(hs, f32)

    in_maps = []
    for c in range(M):
        sl = slice(c * BL, (c + 1) * BL)
        m = {
            "xS": sub_pack(xf[sl]),
            "hS": sub_pack(hf[sl]),
            "hsS": hs_pack(hsf[:, sl]),
            "wpack": wpack,
            "biasp": biasp,
        }
        in_maps.append(m)
    return in_maps


def _run(inputs, trace=False, **trace_kwargs):
    global _cached
    if _cached is None:
        _cached = _build()
    nc = _cached
    in_maps = _prep_inputs(**inputs)
    res = run_bass_kernel_spmd(nc, in_maps, list(range(M)), trace=trace,
                               **trace_kwargs)
    out = np.empty((B, H), np.float32)
    for core in range(M):
        # outF [128, 2*BL]: chunk-major blocks, (f, ch, j) inside
        o = np.asarray(res.results[core]["outF"]).astype(np.float32)
        oc = np.empty((BL, 256), np.float32)
        for (start, ns) in CHUNKS:
            cb0 = 2 * start * SW
            blk = o[:, cb0:cb0 + 2 * ns * SW].reshape(128, 2, ns, SW)
            # oc[(start+ch)*SW + j, f*128 + p] = blk[p, f, ch, j]
            oc[start * SW:(start + ns) * SW, :] = (
                blk.transpose(2, 3, 1, 0).reshape(ns * SW, 256))
        out[core * BL:(core + 1) * BL, :] = oc
    return out, res


def kernel(**inputs):
    return _run(inputs)[0]


# revision 37
# speedup vs baseline: 1.0500x; 1.0315x over previous
"""GRU-style GNN message-passing kernel for Trainium2 (8 NeuronCores, SPMD).

Reference computation (per node b, features 256, 8 neighbors):
    xr = x @ Wir.T + bir
    hr_n = hs_n @ Whr.T + bhr
    r_n = sigmoid(xr + hr_n)
    z = sigmoid(x @ Wiz.T + biz + h_sum @ Whz.T + bhz)
    s = sum_n r_n * hs_n
    n = tanh(x @ Win.T + bin + s @ Whn.T + bhn)
    out = (1 - z) * n + z * h_sum

Strategy: data-parallel over the node dim B=32768 across 8 cores (4096
rows each). Per core the 4096 rows are processed as 8 subchunks of 512
grouped into chunks of width [512,512,1024,1024,512,512] -- small edge
chunks shorten the DMA-bound head and the drain tail, wide middle
chunks amortize per-instruction overhead. Everything on-chip is bf16
(fp32 PSUM accumulation) in feature-major layout. Host pre-packs every
subchunk contiguously in HBM so each DMA moves 2-16KB/partition lines.
Engine placement:
  - PE: all linear-layer matmuls in [128,512] PSUM-bank groups; the
    shared (xr + b_r) term is injected into each neighbor's PSUM group
    via an identity matmul; n-gate emits Win matmuls before Whn ones so
    PE has work while the neighbor sum finishes.
  - ACT: sigmoid/tanh (per-feature bias), xr PSUM drain (+b_r).
  - DVE: r*hs products in place in the hs tile and a linear running sum
    (short dependency tail), plus the final combine out = n + z*(h-n),
    all bf16 2x mode, split by feature half to shorten the tail.
Gate work (z, xr) and the first two neighbors of chunk c+1 are emitted
interleaved into chunk c's neighbor loop, so the two 4-bank PSUM slots
rotate without stalling the PE queue.
"""

import sys
import numpy as np
from contextlib import ExitStack

sys.path.insert(0, "/opt/trn_rl_repo")

import ml_dtypes
import concourse.bacc as bacc
import concourse.tile as tile
from concourse import mybir
from concourse.bass_utils import run_bass_kernel_spmd

F32 = mybir.dt.float32
BF16 = mybir.dt.bfloat16
BF_NP = ml_dtypes.bfloat16

N_NEIGH, B, IN, H = 8, 32768, 256, 256
M = 8                    # cores
BL = B // M              # rows per core (4096)
SW = 512                 # subchunk width
NSUB = BL // SW          # 8 subchunks per core
CHUNKS = [(0, 1), (1, 2), (3, 2), (5, 2), (7, 1)]  # (start, n_subs)
NC = len(CHUNKS)

_cached = None  # compiled program, reused across kernel() calls


def _build():
    nc = bacc.Bacc("TRN2", target_bir_lowering=False, debug=False, num_devices=M)

    # per-subchunk packed inputs (see _prep_inputs for layouts)
    xS = nc.dram_tensor("xS", [NSUB, 128, 2 * SW], BF16, kind="ExternalInput").ap()
    hS = nc.dram_tensor("hS", [NSUB, 128, 2 * SW], BF16, kind="ExternalInput").ap()
    hsS = nc.dram_tensor("hsS", [NSUB, 128, 2 * N_NEIGH * SW], BF16,
                         kind="ExternalInput").ap()
    # all bf16 constants in one block: identity [128,128] first, then 6
    # weights x 2 contraction-row blocks of [128,256] each, ordered so the
    # head-critical ones (wiz, whz, wir) come before whr/win/whn
    wpack = nc.dram_tensor("wpack", [128, 128 + 6 * 512], BF16,
                           kind="ExternalInput").ap()
    # bias pack: col f*3+j holds feature-chunk f of (b_r, b_z, b_n)[j]
    biasp = nc.dram_tensor("biasp", [128, 6], F32, kind="ExternalInput").ap()
    # output: chunk-major blocks, (f, ch, j) within each chunk block
    outF = nc.dram_tensor("outF", [128, 2 * BL], BF16, kind="ExternalOutput").ap()

    with tile.TileContext(nc) as tc, ExitStack() as ctx:
        const_pool = ctx.enter_context(tc.tile_pool(name="const", bufs=1))
        x_pool = ctx.enter_context(tc.tile_pool(name="x", bufs=2))
        h_pool = ctx.enter_context(tc.tile_pool(name="h", bufs=2))
        hs_pool = ctx.enter_context(tc.tile_pool(name="hs", bufs=2))
        xr_pool = ctx.enter_context(tc.tile_pool(name="xr", bufs=2))
        z_pool = ctx.enter_context(tc.tile_pool(name="z", bufs=2))
        r_pool = ctx.enter_context(tc.tile_pool(name="r", bufs=4))
        n_pool = ctx.enter_context(tc.tile_pool(name="n", bufs=2))
        d_pool = ctx.enter_context(tc.tile_pool(name="d", bufs=2))
        o_pool = ctx.enter_context(tc.tile_pool(name="o", bufs=2))
        ps_pool = ctx.enter_context(tc.tile_pool(name="ps", bufs=2, space="PSUM"))

        # --- constants: one bf16 block DMA + one small f32 bias DMA ---
        wpk_t = const_pool.tile([128, 128 + 6 * 512], BF16, tag="wpack",
                                name="wpk_t")
        nc.sync.dma_start(out=wpk_t[:, :], in_=wpack[:, :])
        bias_t = const_pool.tile([128, 6], F32, tag="biasp", name="bias_t")
        nc.sync.dma_start(out=bias_t[:, :], in_=biasp[:, :])

        W_ORDER = ("wiz", "whz", "wir", "whr", "win", "whn")

        def wcol(w, cb, f):       # stationary [128,128]: contract block cb, out block f
            # wpack[p, 128 + (wi*2+cb)*256 + m] = W.T[cb*128+p, m]
            base = 128 + (W_ORDER.index(w) * 2 + cb) * 256 + f * 128
            return wpk_t[:, base:base + 128]

        id_t = wpk_t[:, 0:128]

        # live tiles per chunk index
        xt = [None] * NC
        ht = [None] * NC
        hsc = [None] * NC
        xrt = [None] * NC
        zt = [None] * NC

        def nsubs(c):
            return CHUNKS[c][1]

        def dma_in(c, use_scalar=False):
            ns = nsubs(c)
            start = CHUNKS[c][0]
            xt[c] = x_pool.tile([128, 2 * SW * 2], BF16, tag="x", name=f"x_{c}")
            ht[c] = h_pool.tile([128, 2 * SW * 2], BF16, tag="h", name=f"h_{c}")
            hsc[c] = hs_pool.tile([128, 2 * N_NEIGH * SW * 2], BF16, tag="hs",
                                  name=f"hs_{c}")
            for ch in range(ns):
                s = start + ch
                nc.sync.dma_start(out=xt[c][:, ch * 1024:(ch + 1) * 1024],
                                  in_=xS[s])
                (nc.scalar if use_scalar else nc.sync).dma_start(
                    out=ht[c][:, ch * 1024:(ch + 1) * 1024], in_=hS[s])
                # hs subchunk (2MB) split in two for progressive neighbor deps
                eng2 = nc.scalar if use_scalar else nc.sync
                nc.sync.dma_start(
                    out=hsc[c][:, ch * 8192:ch * 8192 + 4096],
                    in_=hsS[s][:, 0:4096])
                eng2.dma_start(
                    out=hsc[c][:, ch * 8192 + 4096:(ch + 1) * 8192],
                    in_=hsS[s][:, 4096:8192])

        def mslice(t, cb, ch):    # moving [128,512]: x/h tiles, (ch, cb, j) layout
            base = ch * 1024 + cb * 512
            return t[:, base:base + 512]

        def hslice(c, n, cb, ch):  # moving [128,512] of neighbor n
            base = ch * 8192 + (n * 2 + cb) * 512
            return hsc[c][:, base:base + 512]

        def q_of(p, cw, f, ch):    # PSUM quarter (f, ch) of a [128, 2*cw] tile
            base = f * cw + ch * 512
            return p[:, base:base + 512]

        def gates_zx(c, warm=False):
            """z-gate and xr for chunk c: matmuls + ACT/DVE drains."""
            ns = nsubs(c)
            cw = ns * SW
            pz = ps_pool.tile([128, 2048], F32, tag="ps", name=f"pz_{c}")
            if warm:
                # head-only: 8 matmuls on the weight pack (the first DMA to
                # land) into pz banks the real z-gate never touches, so the
                # PE is busy+warm (HAM K=8/8) through the chunk-0 hs wait
                for g in range(2):
                    q = pz[:, 1024 + g * 512:1024 + (g + 1) * 512]
                    for k in range(4):
                        nc.tensor.matmul(q, id_t,
                                         wpk_t[:, 128 + k * 512:
                                                128 + (k + 1) * 512],
                                         start=(k == 0), stop=(k == 3))
            for f in range(2):
                for ch in range(ns):
                    q = q_of(pz, cw, f, ch)
                    nc.tensor.matmul(q, wcol("wiz", 0, f), mslice(xt[c], 0, ch),
                                     start=True, stop=False)
                    nc.tensor.matmul(q, wcol("wiz", 1, f), mslice(xt[c], 1, ch),
                                     start=False, stop=False)
                    nc.tensor.matmul(q, wcol("whz", 0, f), mslice(ht[c], 0, ch),
                                     start=False, stop=False)
                    nc.tensor.matmul(q, wcol("whz", 1, f), mslice(ht[c], 1, ch),
                                     start=False, stop=True)
            zt[c] = z_pool.tile([128, 2048], BF16, tag="z", name=f"z_{c}")
            for f in range(2):
                nc.scalar.activation(zt[c][:, f * cw:(f + 1) * cw],
                                     pz[:, f * cw:(f + 1) * cw],
                                     mybir.ActivationFunctionType.Sigmoid,
                                     bias=bias_t[:, f * 3 + 1:f * 3 + 2])

            pxr = ps_pool.tile([128, 2048], F32, tag="ps", name=f"pxr_{c}")
            for f in range(2):
                for ch in range(ns):
                    q = q_of(pxr, cw, f, ch)
                    nc.tensor.matmul(q, wcol("wir", 0, f), mslice(xt[c], 0, ch),
                                     start=True, stop=False)
                    nc.tensor.matmul(q, wcol("wir", 1, f), mslice(xt[c], 1, ch),
                                     start=False, stop=True)
            xrt[c] = xr_pool.tile([128, 2048], BF16, tag="xr", name=f"xr_{c}")
            # drain on DVE (has slack) instead of ACT: keeps the ACT queue
            # free for the sigmoid cadence at gate boundaries
            with nc.allow_low_precision(reason="bf16 xr"):
                for f in range(2):
                    nc.vector.tensor_scalar_add(xrt[c][:, f * cw:(f + 1) * cw],
                                                pxr[:, f * cw:(f + 1) * cw],
                                                bias_t[:, f * 3:f * 3 + 1])

        def neigh(c, n):
            """r_n = sigmoid(Whr@hs_n + xr), product into hsc, running sum."""
            ns = nsubs(c)
            cw = ns * SW
            pr = ps_pool.tile([128, 2048], F32, tag="ps", name=f"pr{n}_{c}")
            for f in range(2):
                for ch in range(ns):
                    q = q_of(pr, cw, f, ch)
                    nc.tensor.matmul(q, wcol("whr", 0, f), hslice(c, n, 0, ch),
                                     start=True, stop=False)
                    nc.tensor.matmul(q, wcol("whr", 1, f), hslice(c, n, 1, ch),
                                     start=False, stop=False)
                    nc.tensor.matmul(q, id_t,
                                     xrt[c][:, f * cw + ch * 512:
                                            f * cw + ch * 512 + 512],
                                     start=False, stop=True)
            rc = r_pool.tile([128, 2048], BF16, tag="r", name=f"r{n}_{c}")
            nc.scalar.activation(rc[:, 0:2 * cw], pr[:, 0:2 * cw],
                                 mybir.ActivationFunctionType.Sigmoid)
            # rc is (f, ch, j); hs neighbor block is (ch, cb, j): use 4D APs
            rc4 = rc[:, 0:2 * cw].rearrange("p (f ch j) -> p ch f j",
                                            f=2, ch=ns)
            hs4 = hsc[c][:, 0:ns * 8192].rearrange(
                "p (ch nn cb j) -> p nn ch cb j", ch=ns, nn=N_NEIGH, cb=2)
            hs_n = hs4[:, n]
            nc.vector.tensor_mul(hs_n, rc4, hs_n)
            if n > 0:
                acc = hs4[:, 0]
                with nc.allow_low_precision(reason="bf16 neighbor sums"):
                    nc.vector.tensor_add(acc, acc, hs_n)

        def ngate(c):
            """n gate + combine + output DMA for chunk c."""
            ns = nsubs(c)
            cw = ns * SW
            colbase = 2 * CHUNKS[c][0] * SW
            pn = ps_pool.tile([128, 2048], F32, tag="ps", name=f"pn_{c}")
            # all Win matmuls first: they only need x, so PE has work while
            # the DVE accumulation of s finishes
            for f in range(2):
                for ch in range(ns):
                    q = q_of(pn, cw, f, ch)
                    nc.tensor.matmul(q, wcol("win", 0, f), mslice(xt[c], 0, ch),
                                     start=True, stop=False)
                    nc.tensor.matmul(q, wcol("win", 1, f), mslice(xt[c], 1, ch),
                                     start=False, stop=False)
            for f in range(2):
                for ch in range(ns):
                    q = q_of(pn, cw, f, ch)
                    nc.tensor.matmul(q, wcol("whn", 0, f), hslice(c, 0, 0, ch),
                                     start=False, stop=False)
                    nc.tensor.matmul(q, wcol("whn", 1, f), hslice(c, 0, 1, ch),
                                     start=False, stop=True)
            nt = n_pool.tile([128, 2048], BF16, tag="n", name=f"n_{c}")
            dt_ = d_pool.tile([128, 2048], BF16, tag="d", name=f"d_{c}")
            ot = o_pool.tile([128, 2048], BF16, tag="o", name=f"o_{c}")
            # per f-half: tanh then bf16 combine out = n + z*(h - n) on DVE,
            # then its output DMA -- halves the exposed tail latency.
            # h is (ch, f, j)-major, the rest (f, ch, j): rearranged APs.
            for f in range(2):
                fs = slice(f * cw, (f + 1) * cw)
                nc.scalar.activation(nt[:, fs], pn[:, fs],
                                     mybir.ActivationFunctionType.Tanh,
                                     bias=bias_t[:, f * 3 + 2:f * 3 + 3])
                htf = ht[c][:, 0:2 * cw].rearrange(
                    "p (ch f j) -> p f ch j", ch=ns, f=2)[:, f]
                ntf = nt[:, fs].rearrange("p (ch j) -> p ch j", ch=ns)
                dtf = dt_[:, fs].rearrange("p (ch j) -> p ch j", ch=ns)
                ztf = zt[c][:, fs].rearrange("p (ch j) -> p ch j", ch=ns)
                otf = ot[:, fs].rearrange("p (ch j) -> p ch j", ch=ns)
                with nc.allow_low_precision(reason="bf16 combine"):
                    nc.vector.tensor_sub(dtf, htf, ntf)
                    nc.vector.tensor_mul(dtf, ztf, dtf)
                    nc.vector.tensor_add(otf, ntf, dtf)
                nc.sync.dma_start(
                    out=outF[:, colbase + f * cw:colbase + (f + 1) * cw],
                    in_=ot[:, fs])

        # --- software-pipelined emission ---
        dma_in(0, use_scalar=True)
        gates_zx(0, warm=True)
        neigh(0, 0)
        neigh(0, 1)
        for c in range(NC):
            neigh(c, 2)
            neigh(c, 3)
            neigh(c, 4)
            neigh(c, 5)
            if c + 1 < NC:
                dma_in(c + 1)
                gates_zx(c + 1)
            neigh(c, 6)
            neigh(c, 7)
            if c + 1 < NC:
                neigh(c + 1, 0)
            ngate(c)
            if c + 1 < NC:
                neigh(c + 1, 1)

    nc.compile()
    return nc


def _prep_inputs(x, h_sum, hs, Wir, bir, Whr, bhr, Wiz, biz, Whz, bhz,
                 Win, bin_, Whn, bhn):
    """Shard + transpose + subchunk-pack to per-core input maps (all bf16)."""
    f32 = np.float32

    def sub_pack(a):  # [BL, 256] -> [NSUB, 128, 1024]; [s,p,cb*512+j]=a[s*512+j,cb*128+p]
        return np.ascontiguousarray(
            a.reshape(NSUB, SW, 2, 128).transpose(0, 3, 2, 1)
        ).reshape(NSUB, 128, 2 * SW).astype(BF_NP)

    def hs_pack(a):  # [8, BL, 256] -> [NSUB, 128, 8192]; [s,p,(n*2+cb)*512+j]
        return np.ascontiguousarray(
            a.reshape(N_NEIGH, NSUB, SW, 2, 128).transpose(1, 4, 0, 3, 2)
        ).reshape(NSUB, 128, 2 * N_NEIGH * SW).astype(BF_NP)

    # wpack[p, 128 + (wi*2+cb)*256 + m] = W.T[cb*128+p, m]; identity first
    wpack = np.empty((128, 128 + 6 * 512), BF_NP)
    wpack[:, 0:128] = np.eye(128, dtype=f32).astype(BF_NP)
    for wi, Wm in enumerate((Wiz, Whz, Wir, Whr, Win, Whn)):
        Wt = np.asarray(Wm, f32).T.astype(BF_NP)
        wpack[:, 128 + (wi * 2) * 256:128 + (wi * 2 + 1) * 256] = Wt[0:128, :]
        wpack[:, 128 + (wi * 2 + 1) * 256:128 + (wi * 2 + 2) * 256] = Wt[128:256, :]

    b_r = np.asarray(bir, f32) + np.asarray(bhr, f32)
    b_z = np.asarray(biz, f32) + np.asarray(bhz, f32)
    b_n = np.asarray(bin_, f32) + np.asarray(bhn, f32)
    biasp = np.empty((128, 6), f32)
    for f in range(2):
        biasp[:, f * 3 + 0] = b_r[f * 128:(f + 1) * 128]
        biasp[:, f * 3 + 1] = b_z[f * 128:(f + 1) * 128]
        biasp[:, f * 3 + 2] = b_n[f * 128:(f + 1) * 128]

    xf = np.asarray(x, f32)
    hf = np.asarray(h_sum, f32)
    hsf = np.asarray(hs, f32)

    in_maps = []
    for c in range(M):
        sl = slice(c * BL, (c + 1) * BL)
        m = {
            "xS": sub_pack(xf[sl]),
            "hS": sub_pack(hf[sl]),
            "hsS": hs_pack(hsf[:, sl]),
            "wpack": wpack,
            "biasp": biasp,
        }
        in_maps.append(m)
    return in_maps


def _run(inputs, trace=False, **trace_kwargs):
    global _cached
    if _cached is None:
        _cached = _build()
    nc = _cached
    in_maps = _prep_inputs(**inputs)
    res = run_bass_kernel_spmd(nc, in_maps, list(range(M)), trace=trace,
                               **trace_kwargs)
    out = np.empty((B, H), np.float32)
    for core in range(M):
        # outF [128, 2*BL]: chunk-major blocks, (f, ch, j) inside
        o = np.asarray(res.results[core]["outF"]).astype(np.float32)
        oc = np.empty((BL, 256), np.float32)
        for (start, ns) in CHUNKS:
            cb0 = 2 * start * SW
            blk = o[:, cb0:cb0 + 2 * ns * SW].reshape(128, 2, ns, SW)
            # oc[(start+ch)*SW + j, f*128 + p] = blk[p, f, ch, j]
            oc[start * SW:(start + ns) * SW, :] = (
                blk.transpose(2, 3, 1, 0).reshape(ns * SW, 256))
        out[core * BL:(core + 1) * BL, :] = oc
    return out, res


def kernel(**inputs):
    return _run(inputs)[0]
